# revision 13
# baseline (speedup 1.0000x reference)
"""DerivativeNet (direction='x') on 8 Trainium2 NeuronCores.

Contract: kernel(u, mask) takes FULL inputs
  u    [16, 2, 1024, 1024] f32
  mask [16, 1, 1024, 1024] f32
returns FULL output [16, 2, 1024, 1024] f32.

Sharding: pure data-parallel over batch — 2 samples per core, 8 cores.

Math per row along W (h = 0.01, zero-padded):
  d[k]   = up[k+1] - up[k]              (up = [0, u, 0])
  out[w] = p'[w]*d[w+1] + q'[w]*d[w]
  p' = 100*(cs==1) + 50*eroded
  q' = 100*((cs==total)&m) + 50*eroded
  eroded = (cs[w+1]-cs[w-2] == 3),  cs = cumsum(m) along w

Optimized datapath (v2):
  - The whole u path runs in fp16 at DVE 2x: u is cast f32->f16 on the
    ACT engine with scale=50 folded in, so the mask coefficients become
    small ints pco = 2*(cs==1)+er, qco = 2*edge2+er (exact in fp16).
  - The final add t1+t2 runs on the idle PE as two fp16 identity
    matmuls accumulating in PSUM (f32), and the PSUM->SBUF copy on ACT
    doubles as the f32 upcast for the store.
  - DMA is ring-split: loads on the SP(sync) queue, stores on the
    Pool(gpsimd) queue (25ns issue, engine otherwise idle).
  - Mask pipeline stays fp16 (integers exact to 2048 >= W).
"""

import sys

if "/opt/trn_rl_repo" not in sys.path:
    sys.path.insert(0, "/opt/trn_rl_repo")

import numpy as np

_B, _C, _H, _W = 16, 2, 1024, 1024
_NCORES = 8
_BS = _B // _NCORES              # batch per core
_SCALE = 50.0                    # folded 1/(2h); coeffs become 2/1/0

CFG = dict(
    S=2,                 # h-segments per SBUF tile (rows per tile = 128*S)
    bufs=2,              # mask-side tile pool buffers
    ubufs=2,             # u-side tile pool buffers
    fadd="pe",           # final t1+t2 add: "pe" (identity matmul) | "dve"
    er_act=False,        # eroded on ACT via Relu(box-2) instead of DVE ts
    scan_eng="dve",      # cumsum scan engine: "dve" | "gp"
    mset_eng="dve",      # pad/csp memset engine: "dve" | "gp"
    ot_eng="act",        # PSUM->SBUF out copy engine: "act" | "gp"
    pbufs=2,             # PSUM pool buffers (2 tags x 4 banks x pbufs <= 8)
    uload="sync",        # u-load DMA ring
    mload="sync",        # mask-load DMA ring
    store="gp",          # out-store DMA ring
    merge_c=False,       # load both u channels in one 2MiB DMA per group
    iters=1,             # benchmark mode: repeat whole body in a HW loop
    dma_only=False,      # benchmark mode: only DMAs, no compute
    dma_mode="all",      # dma_only sub-mode: "all" | "loads"
)

_CACHE = {}


def _build_nc(cfg=None):
    import concourse.tile as tile
    from concourse import bacc, mybir

    cfg = dict(CFG, **(cfg or {}))
    F32 = mybir.dt.float32
    F16 = mybir.dt.float16
    Alu = mybir.AluOpType

    nc = bacc.Bacc("TRN2", target_bir_lowering=False, debug=False,
                   enable_asserts=False, num_devices=_NCORES)
    u_ap = nc.dram_tensor("u", [_BS, _C, _H, _W], F32,
                          kind="ExternalInput").ap()
    m_ap = nc.dram_tensor("mask", [_BS, _H, _W], F32,
                          kind="ExternalInput").ap()
    o_ap = nc.dram_tensor("out", [_BS, _C, _H, _W], F32,
                          kind="ExternalOutput").ap()

    P, S, W = 128, cfg["S"], _W
    R = P * S
    HT = _H // R
    Wp = W + 4                   # [0,0, cs(0..W-1), tot]
    Wu = W + 2                   # [0, 50u, 0]
    use_pe = cfg["fadd"] == "pe"

    def ring(name):
        return {"sync": nc.sync, "gp": nc.gpsimd, "act": nc.scalar,
                "dve": nc.vector}[cfg[name]]

    with tile.TileContext(nc) as tc:
        with _stack() as ctx:
            pool = ctx.enter_context(tc.tile_pool(name="dn", bufs=cfg["bufs"]))
            upool = ctx.enter_context(tc.tile_pool(name="du",
                                                   bufs=cfg["ubufs"]))
            cpool = ctx.enter_context(tc.tile_pool(name="cn", bufs=1))

            if cfg["er_act"]:
                bias_m2 = cpool.tile([P, 1], F32, tag="bm2")
                nc.vector.memset(bias_m2[:], -2.0)
            if use_pe:
                ppool = ctx.enter_context(
                    tc.tile_pool(name="ps", bufs=cfg["pbufs"], space="PSUM"))
                id_ap = nc.dram_tensor("ident", [P, P], F16,
                                       kind="ExternalInput").ap()
                ident = cpool.tile([P, P], F16, tag="ident")
                nc.sync.dma_start(ident[:], id_ap)

            if cfg["iters"] > 1:
                loop_cm = tc.For_i(0, cfg["iters"], 1)
                ctx.enter_context(loop_cm)

            e_ul, e_ml, e_st = ring("uload"), ring("mload"), ring("store")
            e_sc = ring("scan_eng")
            e_ms = ring("mset_eng")

            if cfg["dma_only"]:
                for b in range(_BS):
                    for ht in range(HT):
                        r0 = ht * R
                        m32 = pool.tile([P, S, W], F32, tag="m32")
                        msrc = m_ap[b, r0:r0 + R, :].rearrange(
                            "(s p) w -> p s w", p=P)
                        e_ml.dma_start(m32[:], msrc)
                        for c in range(_C):
                            up = upool.tile([P, S, W], F32, tag="up32")
                            usrc = u_ap[b, c, r0:r0 + R, :].rearrange(
                                "(s p) w -> p s w", p=P)
                            e_ul.dma_start(up[:], usrc)
                            if cfg["dma_mode"] == "all":
                                odst = o_ap[b, c, r0:r0 + R, :].rearrange(
                                    "(s p) w -> p s w", p=P)
                                e_st.dma_start(odst, up[:])
            for b in ([] if cfg["dma_only"] else range(_BS)):
                for ht in range(HT):
                    r0 = ht * R
                    # ---- mask coefficient pipeline (shared by C channels)
                    m32 = pool.tile([P, S, W], F32, tag="m32")
                    msrc = m_ap[b, r0:r0 + R, :].rearrange(
                        "(s p) w -> p s w", p=P)
                    e_ml.dma_start(m32[:], msrc)
                    mf = pool.tile([P, S, W], F16, tag="mf")
                    nc.scalar.copy(mf[:], m32[:])

                    csp = pool.tile([P, S, Wp], F16, tag="csp")
                    e_ms.memset(csp[:, :, 0:2], 0.0)
                    for s in range(S):
                        e_sc.tensor_tensor_scan(
                            csp[:, s, 2:2 + W], mf[:, s, :], mf[:, s, :],
                            0.0, Alu.add, Alu.bypass)
                    nc.scalar.copy(csp[:, :, 2 + W:3 + W],
                                   csp[:, :, 1 + W:2 + W])

                    cs = csp[:, :, 2:2 + W]
                    box = pool.tile([P, S, W], F16, tag="box")
                    nc.vector.tensor_sub(box[:], csp[:, :, 3:3 + W],
                                         csp[:, :, 0:W])
                    er = pool.tile([P, S, W], F16, tag="er")
                    if cfg["er_act"]:
                        nc.scalar.activation(
                            er[:], box[:],
                            mybir.ActivationFunctionType.Relu,
                            bias=bias_m2[:])
                    else:
                        nc.vector.tensor_scalar(er[:], box[:], 2.5, None,
                                                Alu.is_ge)
                    pco = pool.tile([P, S, W], F16, tag="pco")
                    nc.vector.tensor_scalar(pco[:], cs, 1.0, 2.0,
                                            Alu.is_equal, Alu.mult)
                    nc.vector.tensor_add(pco[:], pco[:], er[:])

                    tot32 = pool.tile([P, S, 1], F32, tag="tot32")
                    nc.scalar.copy(tot32[:], csp[:, :, 1 + W:2 + W])
                    qco = pool.tile([P, S, W], F16, tag="qco")
                    for s in range(S):
                        nc.vector.tensor_scalar(
                            qco[:, s, :], csp[:, s, 2:2 + W],
                            tot32[:, s, :], 2.0,
                            Alu.is_equal, Alu.mult)
                    nc.vector.tensor_mul(qco[:], qco[:], mf[:])
                    nc.vector.tensor_add(qco[:], qco[:], er[:])

                    # ---- u stencil pipeline, per channel (chained:
                    # this emission order pipelines best on the in-order
                    # engine queues; phased variants measured 3.5x worse)
                    if cfg["merge_c"]:
                        up32b = upool.tile([P, _C, S, W], F32, tag="up32b")
                        usrcb = u_ap[b, :, r0:r0 + R, :].rearrange(
                            "c (s p) w -> p c s w", p=P)
                        e_ul.dma_start(up32b[:], usrcb)
                    for c in range(_C):
                        if cfg["merge_c"]:
                            up32 = up32b[:, c]
                        else:
                            up32t = upool.tile([P, S, W], F32, tag="up32")
                            usrc = u_ap[b, c, r0:r0 + R, :].rearrange(
                                "(s p) w -> p s w", p=P)
                            e_ul.dma_start(up32t[:], usrc)
                            up32 = up32t[:]
                        usf = upool.tile([P, S, Wu], F16, tag="usf")
                        nc.scalar.activation(
                            usf[:, :, 1:1 + W], up32,
                            mybir.ActivationFunctionType.Copy,
                            scale=_SCALE)
                        e_ms.memset(usf[:, :, 0:1], 0.0)
                        e_ms.memset(usf[:, :, W + 1:W + 2], 0.0)
                        d = upool.tile([P, S, Wu], F16, tag="d")
                        nc.vector.tensor_sub(d[:, :, 0:W + 1],
                                             usf[:, :, 1:W + 2],
                                             usf[:, :, 0:W + 1])
                        t1 = upool.tile([P, S, W], F16, tag="t1")
                        nc.vector.tensor_mul(t1[:], pco[:],
                                             d[:, :, 1:1 + W])
                        t2 = upool.tile([P, S, W], F16, tag="t2")
                        nc.vector.tensor_mul(t2[:], qco[:],
                                             d[:, :, 0:W])
                        odst = o_ap[b, c, r0:r0 + R, :].rearrange(
                            "(s p) w -> p s w", p=P)
                        ot = upool.tile([P, S, W], F32, tag="ot")
                        if use_pe:
                            pt = ppool.tile([P, S, W], F32, tag="pt")
                            for s in range(S):
                                for j in range(0, W, 512):
                                    nc.tensor.matmul(
                                        pt[:, s, j:j + 512], ident[:],
                                        t1[:, s, j:j + 512],
                                        start=True, stop=False)
                                    nc.tensor.matmul(
                                        pt[:, s, j:j + 512], ident[:],
                                        t2[:, s, j:j + 512],
                                        start=False, stop=True)
                            if cfg["ot_eng"] == "gp":
                                nc.gpsimd.tensor_copy(ot[:], pt[:])
                            else:
                                nc.scalar.copy(ot[:], pt[:])
                        else:
                            nc.vector.tensor_add(t1[:], t1[:], t2[:])
                            nc.scalar.copy(ot[:], t1[:])
                        e_st.dma_start(odst, ot[:])
    nc.compile()
    return nc


def _stack():
    from contextlib import ExitStack
    return ExitStack()


def _get_runner():
    """Build, compile and jit once; return a callable
    (u_full, mask_full) -> out_full that just executes."""
    if "runner" in _CACHE:
        return _CACHE["runner"]

    import jax
    from jax.sharding import Mesh, PartitionSpec
    from jax.experimental.shard_map import shard_map
    from concourse import bass2jax, mybir

    nc = _build_nc()
    bass2jax.install_neuronx_cc_hook()

    partition_name = (nc.partition_id_tensor.name
                      if nc.partition_id_tensor else None)
    in_names = []
    out_names = []
    out_avals = []
    zero_shapes = []
    for alloc in nc.m.functions[0].allocations:
        if not isinstance(alloc, mybir.MemoryLocationSet):
            continue
        name = alloc.memorylocations[0].name
        if alloc.kind == "ExternalInput":
            if name != partition_name:
                in_names.append(name)
        elif alloc.kind == "ExternalOutput":
            out_names.append(name)
            shape = tuple(alloc.tensor_shape)
            dtype = mybir.dt.np(alloc.dtype)
            out_avals.append(jax.core.ShapedArray(shape, dtype))
            zero_shapes.append((shape, dtype))
    n_params = len(in_names)
    all_names = in_names + out_names
    if partition_name is not None:
        all_names = all_names + [partition_name]

    def _body(*args):
        operands = list(args)
        if partition_name is not None:
            operands.append(bass2jax.partition_id_tensor())
        outs = bass2jax._bass_exec_p.bind(
            *operands,
            out_avals=tuple(out_avals),
            in_names=tuple(all_names),
            out_names=tuple(out_names),
            lowering_input_output_aliases=(),
            sim_require_finite=True,
            sim_require_nnan=True,
            nc=nc,
        )
        return tuple(outs)

    devices = jax.devices()[:_NCORES]
    mesh = Mesh(np.asarray(devices), ("core",))
    n_outs = len(out_names)
    sharded = jax.jit(
        shard_map(_body, mesh=mesh,
                  in_specs=(PartitionSpec("core"),) * (n_params + n_outs),
                  out_specs=(PartitionSpec("core"),) * n_outs,
                  check_rep=False),
        donate_argnums=tuple(range(n_params, n_params + n_outs)),
        keep_unused=True,
    )

    name_to_idx = {n: i for i, n in enumerate(in_names)}

    def run(u_full, mask_full):
        u_full = np.ascontiguousarray(u_full, dtype=np.float32)
        mask_full = np.ascontiguousarray(
            mask_full, dtype=np.float32).reshape(_B, _H, _W)
        # per-core shard along axis 0 = declared per-core shape, so the
        # [16, ...] batch-major arrays are already the global view
        args = [None] * n_params
        args[name_to_idx["u"]] = u_full
        args[name_to_idx["mask"]] = mask_full
        if "ident" in name_to_idx:
            args[name_to_idx["ident"]] = np.tile(
                np.eye(128, dtype=np.float16), (_NCORES, 1))
        zeros = [np.zeros((_NCORES * s[0], *s[1:]), d)
                 for (s, d) in zero_shapes]
        out_arrs = sharded(*args, *zeros)
        out = np.asarray(out_arrs[out_names.index("out")])
        return out.reshape(_B, _C, _H, _W)

    _CACHE["runner"] = run
    return run


def kernel(u, mask):
    run = _get_runner()
    return run(u, mask)


def _np_reference(u, mask):
    """Numpy oracle for quick self-checks (mirrors reference.py)."""
    h = 0.01
    up = np.pad(u, ((0, 0), (0, 0), (0, 0), (1, 1)))
    internal_d = (up[..., 2:] - up[..., :-2]) / (2.0 * h)
    left_d = (up[..., 2:] - up[..., 1:-1]) / h
    right_d = (up[..., 1:-1] - up[..., :-2]) / h
    mp = np.pad(mask, ((0, 0), (0, 0), (0, 0), (1, 1)))
    box = mp[..., 2:] + mp[..., 1:-1] + mp[..., :-2]
    eroded = (box == 3.0).astype(u.dtype)
    mb = mask.astype(bool)
    cs = np.cumsum(mb.astype(u.dtype), axis=-1)
    edge1 = (cs == 1.0).astype(u.dtype)
    row_max = np.max(cs, axis=-1, keepdims=True)
    edge2 = ((cs == row_max) & mb).astype(u.dtype)
    return eroded * internal_d + edge1 * left_d + edge2 * right_d


if __name__ == "__main__":
    rng = np.random.default_rng(0)
    u = rng.standard_normal((_B, _C, _H, _W), dtype=np.float32)
    mask = (rng.random((_B, 1, _H, _W)) < 0.5).astype(np.float32)
    out = kernel(u=u, mask=mask)
    exp = _np_reference(u, mask)
    err = np.abs(out - exp)
    denom = float(np.abs(exp).max())
    print("out", out.shape, out.dtype, float(np.abs(out).max()))
    print(f"rel err vs numpy ref: {err.max() / denom:.3e}")
